# revision 14
# baseline (speedup 1.0000x reference)
"""GATv2Conv on 8 Trainium2 NeuronCores — edge-parallel, dst-sharded.

Strategy (per spec sharding_hint, edge parallelism variant):
  * Host sorts edges by dst and splits them into 8 contiguous dst-node
    ranges with ~equal edge counts.  Each core owns a disjoint set of
    destination nodes, so softmax stats and output aggregation are fully
    local — no collectives at all.
  * Each core (SPMD, one NEFF):
      Phase 1: xl = x @ W_l for ALL nodes (replicated compute),
               xr = x @ W_r for its own dst range; both stored as bf16
               rows in a DRAM scratch tensor `xcat`.
      Phase 2: edges are processed in "windows" of <=2048 edge slots
               whose dst span <128 nodes.  Per window:
                 - dma_gather pulls xl[src] / xr[dst] rows (256B bf16)
                   into SBUF.  dma_gather indices are int16, so xl is
                   split into 4 node-quarters and each window's edges
                   are grouped by src quarter on the host (slots padded
                   per quarter to a multiple of 128),
                 - PE transposes u,v into PSUM (z^T = u^T + v^T via
                   accumulating identity matmuls),
                 - scores: LeakyReLU via the exact identity
                   leaky(z) = 0.6 z + 0.4 |z| folded into two PE
                   matmuls against 0.6*att / 0.4*att, ScalarE exp -> p,
                 - a one-hot "scatter matrix" S[e, n] = (dst_rel == n)
                   built on VectorE turns segment-sum into PE matmuls:
                   out[n,f] += S^T @ (u * p),  den[n,h] += S^T @ p,
                 - finalize: out = out/(den+eps) + bias, then
                   dma_scatter_add writes the 128 node rows into the
                   core's (pre-zeroed) output.
  * Host concatenates the 8 per-core row ranges and fixes zero-degree
    rows to `bias` (softmax over an empty segment).

No max-subtraction is needed in the softmax: scores are O(+-10) and all
accumulation happens in fp32 PSUM, so exp() is safely in range.
"""

import numpy as np
import ml_dtypes

import concourse.bass as bass
import concourse.bacc as bacc
import concourse.mybir as mybir
import concourse.tile as tile
from concourse import library_config

BF16 = ml_dtypes.bfloat16

H, C, F, D = 4, 32, 128, 128  # heads, channels/head, H*C, input dim
NCORES = 8
P = 128                       # partitions
EPW = 2048                    # edge slots per window (16 tiles of 128)
SLOTS = EPW // P              # 16
NSUP = EPW // 512             # 4 super-tiles (512 edges) per window
NQ = 4                        # src-quarter split (dma_gather idx is int16)
DUMMY_COL = 999.0             # dst_rel sentinel -> one-hot never matches
EPS = 1e-16
GATHER_CAP = 1024             # dma_gather crashes (ucode) above ~1024 idxs


def _round_up(a, b):
    return (a + b - 1) // b * b


def _wrap16(flat, width):
    """int16 idx layout for dma_gather/scatter: value j at
    [j%16, j//16], replicated across the 8 Q7 core groups."""
    tmp = np.zeros(width * 16, np.int16)
    tmp[:len(flat)] = flat
    return np.tile(tmp.reshape(width, 16).T, (8, 1))


# ----------------------------------------------------------------- host prep


def _split_cores(sdst, n_nodes, n_edges):
    eb = [0]
    nb = [0]
    for c in range(1, NCORES):
        pos = min(n_edges - 1, (n_edges * c) // NCORES)
        node = int(sdst[pos])
        eb.append(int(np.searchsorted(sdst, node)))
        nb.append(node)
    eb.append(n_edges)
    nb.append(n_nodes)
    return eb, nb


QCAP = EPW // NQ  # 512 slots per fixed src-quarter region


def _build_core_windows(ssrc_c, sdst_c, nb_c, nr_c, qrows):
    """Pack one core's (dst-sorted) edges into fixed 2048-slot windows
    with four FIXED 512-slot src-quarter regions (slot layout is static,
    identical across cores).  A window closes when any quarter region is
    full or the dst span would reach 128 nodes."""
    ne = len(ssrc_c)
    if ne == 0:
        nodes = np.zeros(0, np.int64)
        counts = np.zeros(0, np.int64)
        seg_of_edge = np.zeros(0, np.int64)
    else:
        change = np.flatnonzero(np.diff(sdst_c)) + 1
        starts = np.concatenate(([0], change))
        nodes = sdst_c[starts]
        counts = np.diff(np.concatenate((starts, [ne])))
        seg_of_edge = np.repeat(np.arange(len(nodes)), counts)

    equarter = ssrc_c // qrows  # [ne]
    nseg = len(nodes)
    segq = np.zeros((nseg, NQ), np.int64)
    if ne:
        np.add.at(segq, (seg_of_edge, equarter), 1)
    assert nseg == 0 or segq.max() <= QCAP, "per-quarter degree too big"

    win_segs = []  # (first_seg, one_past_last_seg)
    i = 0
    while i < nseg:
        base = nodes[i]
        qc = np.zeros(NQ, np.int64)
        j = i
        while j < nseg and nodes[j] - base < P and (qc + segq[j]).max() <= QCAP:
            qc += segq[j]
            j += 1
        assert j > i, "single segment does not fit a window"
        win_segs.append((i, j))
        i = j
    wc = len(win_segs)

    uidx = np.zeros((wc, P, SLOTS), np.int16)
    vidx = np.zeros((wc, P, SLOTS), np.int16)
    dstrel = np.full((wc, P, SLOTS), DUMMY_COL, np.float32)
    win_nb = np.zeros(wc, np.int64)
    own_end = np.zeros(wc, np.int64)

    for w, (si, sj) in enumerate(win_segs):
        win_nb[w] = nodes[si]
        e0 = int(np.searchsorted(seg_of_edge, si))
        e1 = int(np.searchsorted(seg_of_edge, sj - 1, side="right"))
        es = ssrc_c[e0:e1]
        ed = sdst_c[e0:e1]
        eq = equarter[e0:e1]
        for q in range(NQ):
            sel = eq == q
            cq = int(sel.sum())
            if cq:
                slots = q * QCAP + np.arange(cq)
                pp = slots % P
                ss = slots // P
                uidx[w, pp, ss] = (es[sel] - q * qrows).astype(np.int16)
                vidx[w, pp, ss] = (ed[sel] - nb_c).astype(np.int16)
                dstrel[w, pp, ss] = (ed[sel] - win_nb[w]).astype(np.float32)
        own_end[w] = nodes[sj] if sj < nseg else nb_c + nr_c
        own_end[w] = min(own_end[w], win_nb[w] + P)

    return dict(win_nb=win_nb, own_end=own_end, uidx=uidx, vidx=vidx,
                dstrel=dstrel, wc=wc)


# ------------------------------------------------------------- bass program


def _build_nc(W, npad_xl, nrx, nr_pad):
    """Per-core SPMD bass program (fixed 4x512 quarter slot layout)."""
    nc = bacc.Bacc("TRN2", target_bir_lowering=False, debug=False)
    bf = mybir.dt.bfloat16
    f32 = mybir.dt.float32
    i16 = mybir.dt.int16
    qrows = npad_xl // NQ

    xT = nc.dram_tensor("xT", [P, npad_xl], bf, kind="ExternalInput")
    xrT = nc.dram_tensor("xrT", [P, nrx], bf, kind="ExternalInput")
    Wl = nc.dram_tensor("Wl", [P, F], bf, kind="ExternalInput")
    Wr = nc.dram_tensor("Wr", [P, F], bf, kind="ExternalInput")
    # attm[:, :H] = 0.6*attmat, attm[:, H:] = 0.4*attmat (leaky identity)
    attm = nc.dram_tensor("attm", [P, 2 * H], bf, kind="ExternalInput")
    bias_bc = nc.dram_tensor("bias_bc", [P, F], f32, kind="ExternalInput")
    # per-window int16 idx planes (wrapped layout, 16 values/column):
    # cols [0:128)=u, [128:256)=v, [256:264)=scatter rows
    UW = EPW // 16
    gidx = nc.dram_tensor("gidx", [W, P, 2 * UW + 8], i16,
                          kind="ExternalInput")
    drel = nc.dram_tensor("drel", [W, P, SLOTS], bf, kind="ExternalInput")

    outp = nc.dram_tensor("outp", [nr_pad + P, F], f32, kind="ExternalOutput")
    xcat = nc.dram_tensor("xcat", [npad_xl + nrx, F], bf, kind="Internal")

    ident_np = np.eye(P, dtype=np.float32).astype(BF16)
    iota_np = np.tile(np.arange(P, dtype=np.float32), (P, NSUP)).astype(BF16)
    ident_d = nc.inline_tensor(ident_np, name="ident")
    iota_d = nc.inline_tensor(iota_np, name="iota4")

    Abs = mybir.ActivationFunctionType.Abs
    Exp = mybir.ActivationFunctionType.Exp

    with tile.TileContext(nc) as tc:
        with tc.tile_pool(name="const", bufs=1) as cpool:
            nc.gpsimd.load_library(library_config.mlp)
            ident_sb = cpool.tile([P, P], bf, tag="ident")
            nc.sync.dma_start(out=ident_sb[:], in_=ident_d.ap())
            iota_sb = cpool.tile([P, NSUP, P], bf, tag="iota")
            nc.sync.dma_start(
                out=iota_sb[:],
                in_=iota_d.ap().rearrange("p (a b) -> p a b", a=NSUP))
            attm_sb = cpool.tile([P, 2 * H], bf, tag="attm")
            nc.sync.dma_start(out=attm_sb[:], in_=attm[:])
            bias_sb = cpool.tile([P, F], f32, tag="bias")
            nc.sync.dma_start(out=bias_sb[:], in_=bias_bc[:])
            wl_sb = cpool.tile([P, F], bf, tag="wl")
            nc.sync.dma_start(out=wl_sb[:], in_=Wl[:])
            wr_sb = cpool.tile([P, F], bf, tag="wr")
            nc.sync.dma_start(out=wr_sb[:], in_=Wr[:])

            # ---------------- phase 1: xcat = [x @ Wl ; x_range @ Wr] (bf16)
            with (
                tc.tile_pool(name="ph1", bufs=3) as p1,
                tc.tile_pool(name="ph1ps", bufs=2, space="PSUM") as p1ps,
            ):
                def linear_chunks(src_T, w_sb, row0, nchunks):
                    for k in range(nchunks):
                        xt = p1.tile([P, 512], bf, tag="xt")
                        nc.sync.dma_start(
                            out=xt[:], in_=src_T[:, 512 * k: 512 * (k + 1)])
                        ps = p1ps.tile([P, 512], f32, tag="ps1")
                        for j in range(4):
                            nc.tensor.matmul(
                                out=ps[:, P * j: P * (j + 1)],
                                lhsT=xt[:, P * j: P * (j + 1)],
                                rhs=w_sb[:],
                                start=True, stop=True)
                        st = p1.tile([P, 4, F], bf, tag="st")
                        stv = st[:].rearrange("p a b -> p (a b)")
                        if k % 2 == 0:
                            nc.vector.tensor_copy(out=stv, in_=ps[:])
                        else:
                            nc.scalar.copy(out=stv, in_=ps[:])
                        nc.sync.dma_start(
                            out=xcat[row0 + 512 * k: row0 + 512 * (k + 1), :]
                            .rearrange("(j p) f -> p j f", p=P),
                            in_=st[:])

                linear_chunks(xT, wl_sb, 0, npad_xl // 512)
                linear_chunks(xrT, wr_sb, npad_xl, nrx // 512)

            # ---------------- phase 2: edge windows
            with (
                tc.tile_pool(name="win", bufs=3) as wp,
                tc.tile_pool(name="gat", bufs=2) as gp,
                tc.tile_pool(name="mid", bufs=3) as mp,
                tc.tile_pool(name="fin", bufs=2) as fp,
                tc.tile_pool(name="pszt", bufs=2, space="PSUM") as ps_zt,
                tc.tile_pool(name="pssc", bufs=2, space="PSUM") as ps_sc,
                tc.tile_pool(name="psout", bufs=2, space="PSUM") as ps_out,
                tc.tile_pool(name="psden", bufs=2, space="PSUM") as ps_den,
            ):
                for w in range(W):
                    gx = wp.tile([P, 2 * UW + 8], i16, tag="gx")
                    nc.sync.dma_start(out=gx[:], in_=gidx[w])
                    dr = wp.tile([P, SLOTS], bf, tag="dr")
                    nc.sync.dma_start(out=dr[:], in_=drel[w])

                    g = gp.tile([P, 2 * SLOTS, F], bf, tag="g")
                    for q in range(NQ):
                        off = q * QCAP
                        nc.gpsimd.dma_gather(
                            g[:, off // P: (off + QCAP) // P, :],
                            xcat[q * qrows: (q + 1) * qrows, :],
                            gx[:, off // 16: (off + QCAP) // 16],
                            QCAP, QCAP, F)
                    for k in range(EPW // GATHER_CAP):
                        t0 = SLOTS + k * (GATHER_CAP // P)
                        nc.gpsimd.dma_gather(
                            g[:, t0: t0 + GATHER_CAP // P, :],
                            xcat[npad_xl: npad_xl + nrx, :],
                            gx[:, UW + k * (GATHER_CAP // 16):
                               UW + (k + 1) * (GATHER_CAP // 16)],
                            GATHER_CAP, GATHER_CAP, F)

                    # scores
                    pp = ps_sc.tile([P, SLOTS * H], f32, tag="pp")
                    for s in range(NSUP):
                        zt = ps_zt.tile([P, 512], f32, tag="zt")
                        for t in range(4):
                            e = 4 * s + t
                            nc.tensor.matmul(
                                out=zt[:, P * t: P * (t + 1)],
                                lhsT=g[:, e, :], rhs=ident_sb[:],
                                start=True, stop=False)
                            nc.tensor.matmul(
                                out=zt[:, P * t: P * (t + 1)],
                                lhsT=g[:, SLOTS + e, :], rhs=ident_sb[:],
                                start=False, stop=True)
                        z_sb = mp.tile([P, 512], bf, tag="z_sb")
                        nc.scalar.copy(out=z_sb[:], in_=zt[:])
                        a_sb = mp.tile([P, 512], bf, tag="a_sb")
                        nc.scalar.activation(out=a_sb[:], in_=zt[:], func=Abs)
                        for t in range(4):
                            e = 4 * s + t
                            nc.tensor.matmul(
                                out=pp[:, H * e: H * (e + 1)],
                                lhsT=z_sb[:, P * t: P * (t + 1)],
                                rhs=attm_sb[:, :H], start=True, stop=False)
                            nc.tensor.matmul(
                                out=pp[:, H * e: H * (e + 1)],
                                lhsT=a_sb[:, P * t: P * (t + 1)],
                                rhs=attm_sb[:, H:], start=False, stop=True)
                    psb = mp.tile([P, SLOTS, H], bf, tag="psb")
                    nc.scalar.activation(
                        out=psb[:].rearrange("p a b -> p (a b)"), in_=pp[:],
                        func=Exp)

                    # aggregation
                    po = ps_out.tile([P, F], f32, tag="po")
                    pd = ps_den.tile([P, H], f32, tag="pd")
                    for s in range(NSUP):
                        S4 = mp.tile([P, 4, P], bf, tag="S4")
                        nc.vector.tensor_tensor(
                            out=S4[:], in0=iota_sb[:],
                            in1=dr[:, 4 * s: 4 * s + 4, None]
                            .broadcast_to([P, 4, P]),
                            op=mybir.AluOpType.is_equal)
                        xjp = mp.tile([P, 4, H, C], bf, tag="xjp")
                        nc.vector.tensor_tensor(
                            out=xjp[:],
                            in0=g[:, 4 * s: 4 * s + 4, :]
                            .rearrange("p t (h c) -> p t h c", h=H),
                            in1=psb[:, 4 * s: 4 * s + 4, :, None]
                            .broadcast_to([P, 4, H, C]),
                            op=mybir.AluOpType.mult)
                        xjp_f = xjp[:].rearrange("p t h c -> p t (h c)")
                        for t in range(4):
                            first = s == 0 and t == 0
                            last = s == NSUP - 1 and t == 3
                            nc.tensor.matmul(
                                out=po[:], lhsT=S4[:, t, :], rhs=xjp_f[:, t, :],
                                start=first, stop=last)
                            nc.tensor.matmul(
                                out=pd[:], lhsT=S4[:, t, :],
                                rhs=psb[:, 4 * s + t, :],
                                start=first, stop=last)

                    dn = fp.tile([P, H], f32, tag="dn")
                    nc.vector.tensor_scalar_add(out=dn[:], in0=pd[:],
                                                scalar1=EPS)
                    rd = fp.tile([P, H], f32, tag="rd")
                    nc.vector.reciprocal(out=rd[:], in_=dn[:])
                    fin = fp.tile([P, H, C], f32, tag="fin")
                    nc.vector.tensor_tensor(
                        out=fin[:],
                        in0=po[:].rearrange("p (h c) -> p h c", h=H),
                        in1=rd[:, :, None].broadcast_to([P, H, C]),
                        op=mybir.AluOpType.mult)
                    fin2 = fp.tile([P, 1, F], f32, tag="fin2")
                    nc.vector.tensor_tensor(
                        out=fin2[:, 0, :],
                        in0=fin[:].rearrange("p h c -> p (h c)"),
                        in1=bias_sb[:], op=mybir.AluOpType.add)
                    nc.gpsimd.dma_scatter_add(
                        outp[:], fin2[:], gx[:, 2 * UW: 2 * UW + 8],
                        P, P, F)

    nc.compile()
    return nc


# ------------------------------------------------------------------- driver


def _prepare(x, edge_index, W_l, W_r, att, bias):
    n_nodes = x.shape[0]
    n_edges = edge_index.shape[1]
    src = np.asarray(edge_index[0], np.int64)
    dst = np.asarray(edge_index[1], np.int64)
    order = np.argsort(dst, kind="stable")
    ssrc = src[order]
    sdst = dst[order]

    eb, nb = _split_cores(sdst, n_nodes, n_edges)
    npad_xl = _round_up(n_nodes, 2048)  # divisible by 4 quarters of 512-mult
    qrows = npad_xl // NQ
    assert qrows <= 32768

    cores = []
    for c in range(NCORES):
        nr_c = nb[c + 1] - nb[c]
        assert nr_c < 32000
        cores.append(_build_core_windows(
            ssrc[eb[c]:eb[c + 1]], sdst[eb[c]:eb[c + 1]], nb[c], nr_c, qrows))

    W = max(cd["wc"] for cd in cores)
    nr_pad = max(nb[c + 1] - nb[c] for c in range(NCORES))
    nrx = _round_up(max(nr_pad, 512), 512)

    xb = np.asarray(x, np.float32).astype(BF16)
    xT_np = np.zeros((P, npad_xl), BF16)
    xT_np[:, :n_nodes] = xb.T

    wl_np = np.asarray(W_l, np.float32).astype(BF16)
    wr_np = np.asarray(W_r, np.float32).astype(BF16)
    att_np = np.asarray(att, np.float32)
    attm_np = np.zeros((P, 2 * H), np.float32)
    for h in range(H):
        attm_np[C * h:C * (h + 1), h] = 0.6 * att_np[h]
        attm_np[C * h:C * (h + 1), H + h] = 0.4 * att_np[h]
    attm_np = attm_np.astype(BF16)
    bias_np = np.asarray(bias, np.float32)
    bias_bc_np = np.tile(bias_np[None, :], (P, 1)).astype(np.float32)

    in_maps = []
    for c in range(NCORES):
        cd = cores[c]
        wc = cd["wc"]
        nr_c = nb[c + 1] - nb[c]

        UW = EPW // 16
        gidx_np = np.zeros((W, P, 2 * UW + 8), np.int16)
        drel_np = np.full((W, P, SLOTS), DUMMY_COL, np.float32)
        # scatter rows: default everything (incl. dummy windows) to trash
        trash = _wrap16((nr_pad + np.arange(P)).astype(np.int16), 8)
        gidx_np[:, :, 2 * UW:] = trash[None]

        for w in range(wc):
            # uidx/vidx/dstrel are already [P, SLOTS] slot-shaped
            # (slot j at [j%128, j//128]); rewrap for the int16 planes
            # (value j at [j%16, j//16]).
            uflat = cd["uidx"][w].T.reshape(-1)  # slot order: s*128+p
            vflat = cd["vidx"][w].T.reshape(-1)
            gidx_np[w, :, :UW] = _wrap16(uflat, UW)
            gidx_np[w, :, UW:2 * UW] = _wrap16(vflat, UW)
            drel_np[w] = cd["dstrel"][w]
            rows = (cd["win_nb"][w] - nb[c]) + np.arange(P)
            owned = rows < (cd["own_end"][w] - nb[c])
            rows = np.where(owned, rows, nr_pad + np.arange(P))
            gidx_np[w, :, 2 * UW:] = _wrap16(rows.astype(np.int16), 8)

        xrT_np = np.zeros((P, nrx), BF16)
        xrT_np[:, :nr_c] = xb[nb[c]:nb[c + 1]].T

        in_maps.append({
            "xT": xT_np,
            "xrT": xrT_np,
            "Wl": wl_np,
            "Wr": wr_np,
            "attm": attm_np,
            "bias_bc": bias_bc_np,
            "gidx": gidx_np,
            "drel": drel_np.astype(BF16),
        })

    meta = dict(W=W, npad_xl=npad_xl, nrx=nrx, nr_pad=nr_pad, nb=nb,
                n_nodes=n_nodes, bias=bias_np,
                deg=np.bincount(dst, minlength=n_nodes))
    return in_maps, meta


_last_results = None


def kernel(x, edge_index, W_l, W_r, att, bias, _sim=False, _trace=False):
    global _last_results
    in_maps, meta = _prepare(x, edge_index, W_l, W_r, att, bias)
    nc = _build_nc(meta["W"], meta["npad_xl"], meta["nrx"], meta["nr_pad"])

    if _sim:
        from concourse.bass_interp import CoreSim
        results = []
        for c in range(NCORES):
            sim = CoreSim(nc, trace=False)
            for k, v in in_maps[c].items():
                sim.tensor(k)[:] = v
            sim.tensor("outp")[:] = 0.0
            sim.simulate()
            results.append({"outp": np.array(sim.tensor("outp"))})
    else:
        from concourse import bass_utils
        r = bass_utils.run_bass_kernel_spmd(
            nc, in_maps, core_ids=list(range(NCORES)), trace=_trace)
        _last_results = r
        results = r.results

    n_nodes = meta["n_nodes"]
    nb = meta["nb"]
    out = np.empty((n_nodes, F), np.float32)
    for c in range(NCORES):
        nr_c = nb[c + 1] - nb[c]
        out[nb[c]:nb[c + 1]] = results[c]["outp"][:nr_c]
    out[meta["deg"] == 0] = meta["bias"][None, :]
    return out


# revision 16
# speedup vs baseline: 1.8461x; 1.8461x over previous
"""GATv2Conv on 8 Trainium2 NeuronCores — edge-parallel, dst-sharded.

Strategy (per spec sharding_hint, edge parallelism variant):
  * Host sorts edges by dst and splits them into 8 contiguous dst-node
    ranges with ~equal edge counts.  Each core owns a disjoint set of
    destination nodes, so softmax stats and output aggregation are fully
    local — no collectives at all.
  * Each core (SPMD, one NEFF):
      Phase 1: xl = x @ W_l for ALL nodes (replicated compute),
               xr = x @ W_r for its own dst range; both stored as bf16
               rows in a DRAM scratch tensor `xcat`.
      Phase 2: edges are processed in "windows" of <=2048 edge slots
               whose dst span <128 nodes.  Per window:
                 - dma_gather pulls xl[src] / xr[dst] rows (256B bf16)
                   into SBUF.  dma_gather indices are int16, so xl is
                   split into 4 node-quarters and each window's edges
                   are grouped by src quarter on the host (slots padded
                   per quarter to a multiple of 128),
                 - PE transposes u,v into PSUM (z^T = u^T + v^T via
                   accumulating identity matmuls),
                 - scores: LeakyReLU via the exact identity
                   leaky(z) = 0.6 z + 0.4 |z| folded into two PE
                   matmuls against 0.6*att / 0.4*att, ScalarE exp -> p,
                 - a one-hot "scatter matrix" S[e, n] = (dst_rel == n)
                   built on VectorE turns segment-sum into PE matmuls:
                   out[n,f] += S^T @ (u * p),  den[n,h] += S^T @ p,
                 - finalize: out = out/(den+eps) + bias, then
                   dma_scatter_add writes the 128 node rows into the
                   core's (pre-zeroed) output.
  * Host concatenates the 8 per-core row ranges and fixes zero-degree
    rows to `bias` (softmax over an empty segment).

No max-subtraction is needed in the softmax: scores are O(+-10) and all
accumulation happens in fp32 PSUM, so exp() is safely in range.
"""

import numpy as np
import ml_dtypes

import concourse.bass as bass
import concourse.bacc as bacc
import concourse.mybir as mybir
import concourse.tile as tile
from concourse import library_config

BF16 = ml_dtypes.bfloat16

H, C, F, D = 4, 32, 128, 128  # heads, channels/head, H*C, input dim
NCORES = 8
P = 128                       # partitions
EPW = 2048                    # edge slots per window (16 tiles of 128)
SLOTS = EPW // P              # 16
NSUP = EPW // 512             # 4 super-tiles (512 edges) per window
NQ = 4                        # src-quarter split (dma_gather idx is int16)
DUMMY_COL = 999.0             # dst_rel sentinel -> one-hot never matches
EPS = 1e-16
GATHER_CAP = 1024             # dma_gather crashes (ucode) above ~1024 idxs


def _round_up(a, b):
    return (a + b - 1) // b * b


def _wrap16(flat, width):
    """int16 idx layout for dma_gather/scatter: value j at
    [j%16, j//16], replicated across the 8 Q7 core groups."""
    tmp = np.zeros(width * 16, np.int16)
    tmp[:len(flat)] = flat
    return np.tile(tmp.reshape(width, 16).T, (8, 1))


# ----------------------------------------------------------------- host prep


def _split_cores(sdst, n_nodes, n_edges):
    eb = [0]
    nb = [0]
    for c in range(1, NCORES):
        pos = min(n_edges - 1, (n_edges * c) // NCORES)
        node = int(sdst[pos])
        eb.append(int(np.searchsorted(sdst, node)))
        nb.append(node)
    eb.append(n_edges)
    nb.append(n_nodes)
    return eb, nb


QCAP = EPW // NQ  # 512 slots per fixed src-quarter region


def _build_core_windows(ssrc_c, sdst_c, nb_c, nr_c, qrows):
    """Pack one core's (dst-sorted) edges into fixed 2048-slot windows
    with four FIXED 512-slot src-quarter regions (slot layout is static,
    identical across cores).  A window closes when any quarter region is
    full or the dst span would reach 128 nodes."""
    ne = len(ssrc_c)
    if ne == 0:
        nodes = np.zeros(0, np.int64)
        counts = np.zeros(0, np.int64)
        seg_of_edge = np.zeros(0, np.int64)
    else:
        change = np.flatnonzero(np.diff(sdst_c)) + 1
        starts = np.concatenate(([0], change))
        nodes = sdst_c[starts]
        counts = np.diff(np.concatenate((starts, [ne])))
        seg_of_edge = np.repeat(np.arange(len(nodes)), counts)

    equarter = ssrc_c // qrows  # [ne]
    nseg = len(nodes)
    segq = np.zeros((nseg, NQ), np.int64)
    if ne:
        np.add.at(segq, (seg_of_edge, equarter), 1)
    assert nseg == 0 or segq.max() <= QCAP, "per-quarter degree too big"

    win_segs = []  # (first_seg, one_past_last_seg)
    i = 0
    while i < nseg:
        base = nodes[i]
        qc = np.zeros(NQ, np.int64)
        j = i
        while j < nseg and nodes[j] - base < P and (qc + segq[j]).max() <= QCAP:
            qc += segq[j]
            j += 1
        assert j > i, "single segment does not fit a window"
        win_segs.append((i, j))
        i = j
    wc = len(win_segs)

    uidx = np.zeros((wc, P, SLOTS), np.int16)
    vidx = np.zeros((wc, P, SLOTS), np.int16)
    dstrel = np.full((wc, P, SLOTS), DUMMY_COL, np.float32)
    win_nb = np.zeros(wc, np.int64)
    own_end = np.zeros(wc, np.int64)

    for w, (si, sj) in enumerate(win_segs):
        win_nb[w] = nodes[si]
        e0 = int(np.searchsorted(seg_of_edge, si))
        e1 = int(np.searchsorted(seg_of_edge, sj - 1, side="right"))
        es = ssrc_c[e0:e1]
        ed = sdst_c[e0:e1]
        eq = equarter[e0:e1]
        for q in range(NQ):
            sel = eq == q
            cq = int(sel.sum())
            if cq:
                slots = q * QCAP + np.arange(cq)
                pp = slots % P
                ss = slots // P
                uidx[w, pp, ss] = (es[sel] - q * qrows).astype(np.int16)
                vidx[w, pp, ss] = (ed[sel] - nb_c).astype(np.int16)
                dstrel[w, pp, ss] = (ed[sel] - win_nb[w]).astype(np.float32)
        own_end[w] = nodes[sj] if sj < nseg else nb_c + nr_c
        own_end[w] = min(own_end[w], win_nb[w] + P)

    return dict(win_nb=win_nb, own_end=own_end, uidx=uidx, vidx=vidx,
                dstrel=dstrel, wc=wc)


# ------------------------------------------------------------- bass program


def _build_nc(W, npad_xl, nrx, nr_pad):
    """Per-core SPMD bass program (fixed 4x512 quarter slot layout)."""
    nc = bacc.Bacc("TRN2", target_bir_lowering=False, debug=False)
    bf = mybir.dt.bfloat16
    f32 = mybir.dt.float32
    i16 = mybir.dt.int16
    qrows = npad_xl // NQ

    xT = nc.dram_tensor("xT", [P, npad_xl], bf, kind="ExternalInput")
    xrT = nc.dram_tensor("xrT", [P, nrx], bf, kind="ExternalInput")
    Wl = nc.dram_tensor("Wl", [P, F], bf, kind="ExternalInput")
    Wr = nc.dram_tensor("Wr", [P, F], bf, kind="ExternalInput")
    # attm[:, :H] = 0.6*attmat, attm[:, H:] = 0.4*attmat (leaky identity)
    attm = nc.dram_tensor("attm", [P, 2 * H], bf, kind="ExternalInput")
    bias_bc = nc.dram_tensor("bias_bc", [P, F], f32, kind="ExternalInput")
    # per-window int16 u-gather idx (wrapped layout, 16 values/column)
    UW = EPW // 16
    gidx = nc.dram_tensor("gidx", [W, P, UW], i16, kind="ExternalInput")
    drel = nc.dram_tensor("drel", [W, P, SLOTS], bf, kind="ExternalInput")
    # flat per-slot dst_rel in one partition row (for the S^T broadcast)
    drelf = nc.dram_tensor("drelf", [W, 1, EPW], bf, kind="ExternalInput")

    outp = nc.dram_tensor("outp", [W * P, F], f32, kind="ExternalOutput")
    xcat = nc.dram_tensor("xcat", [npad_xl + nrx, F], bf, kind="Internal")

    ident_np = np.eye(P, dtype=np.float32).astype(BF16)
    iota_np = np.tile(np.arange(P, dtype=np.float32), (P, NSUP)).astype(BF16)
    ident_d = nc.inline_tensor(ident_np, name="ident")
    iota_d = nc.inline_tensor(iota_np, name="iota4")
    ones_d = nc.inline_tensor(np.ones((1, P), dtype=np.float32).astype(BF16),
                              name="ones_c")
    iotac_d = nc.inline_tensor(
        np.arange(P, dtype=np.float32).reshape(P, 1).copy(), name="iota_col")

    Abs = mybir.ActivationFunctionType.Abs
    Exp = mybir.ActivationFunctionType.Exp

    with tile.TileContext(nc) as tc:
        with tc.tile_pool(name="const", bufs=1) as cpool:
            nc.gpsimd.load_library(library_config.mlp)
            ident_sb = cpool.tile([P, P], bf, tag="ident")
            nc.sync.dma_start(out=ident_sb[:], in_=ident_d.ap())
            iota_sb = cpool.tile([P, NSUP, P], bf, tag="iota")
            nc.sync.dma_start(
                out=iota_sb[:],
                in_=iota_d.ap().rearrange("p (a b) -> p a b", a=NSUP))
            attm_sb = cpool.tile([P, 2 * H], bf, tag="attm")
            nc.sync.dma_start(out=attm_sb[:], in_=attm[:])
            bias_sb = cpool.tile([P, F], f32, tag="bias")
            nc.sync.dma_start(out=bias_sb[:], in_=bias_bc[:])
            wl_sb = cpool.tile([P, F], bf, tag="wl")
            nc.sync.dma_start(out=wl_sb[:], in_=Wl[:])
            wr_sb = cpool.tile([P, F], bf, tag="wr")
            nc.sync.dma_start(out=wr_sb[:], in_=Wr[:])
            ones_sb = cpool.tile([1, P], bf, tag="ones")
            nc.sync.dma_start(out=ones_sb[:], in_=ones_d.ap())
            iotac_sb = cpool.tile([P, 1], f32, tag="iotac")
            nc.sync.dma_start(out=iotac_sb[:], in_=iotac_d.ap())

            # ---------------- phase 1: xcat = [x @ Wl ; x_range @ Wr] (bf16)
            with (
                tc.tile_pool(name="ph1", bufs=3) as p1,
                tc.tile_pool(name="ph1ps", bufs=2, space="PSUM") as p1ps,
            ):
                def linear_chunks(src_T, w_sb, row0, nchunks):
                    for k in range(nchunks):
                        xt = p1.tile([P, 512], bf, tag="xt")
                        nc.sync.dma_start(
                            out=xt[:], in_=src_T[:, 512 * k: 512 * (k + 1)])
                        ps = p1ps.tile([P, 512], f32, tag="ps1")
                        for j in range(4):
                            nc.tensor.matmul(
                                out=ps[:, P * j: P * (j + 1)],
                                lhsT=xt[:, P * j: P * (j + 1)],
                                rhs=w_sb[:],
                                start=True, stop=True)
                        st = p1.tile([P, 4, F], bf, tag="st")
                        stv = st[:].rearrange("p a b -> p (a b)")
                        if k % 2 == 0:
                            nc.vector.tensor_copy(out=stv, in_=ps[:])
                        else:
                            nc.scalar.copy(out=stv, in_=ps[:])
                        nc.sync.dma_start(
                            out=xcat[row0 + 512 * k: row0 + 512 * (k + 1), :]
                            .rearrange("(j p) f -> p j f", p=P),
                            in_=st[:])

                linear_chunks(xT, wl_sb, 0, npad_xl // 512)
                linear_chunks(xrT, wr_sb, npad_xl, nrx // 512)

            # ---------------- phase 2: edge windows
            with (
                tc.tile_pool(name="win", bufs=3) as wp,
                tc.tile_pool(name="gat", bufs=2) as gp,
                tc.tile_pool(name="mid", bufs=3) as mp,
                tc.tile_pool(name="fin", bufs=2) as fp,
                tc.tile_pool(name="pszt", bufs=2, space="PSUM") as ps_zt,
                tc.tile_pool(name="pssc", bufs=2, space="PSUM") as ps_sc,
                tc.tile_pool(name="psod", bufs=2, space="PSUM") as ps_od,
                tc.tile_pool(name="psdb", bufs=2, space="PSUM") as ps_db,
            ):
                for w in range(W):
                    gx = wp.tile([P, UW], i16, tag="gx")
                    nc.sync.dma_start(out=gx[:], in_=gidx[w])
                    dr = wp.tile([P, SLOTS], bf, tag="dr")
                    nc.sync.dma_start(out=dr[:], in_=drel[w])
                    drf = wp.tile([1, EPW], bf, tag="drf")
                    nc.sync.dma_start(out=drf[:], in_=drelf[w])
                    xr_w = wp.tile([P, F], bf, tag="xr_w")
                    nc.sync.dma_start(
                        out=xr_w[:],
                        in_=xcat[npad_xl + P * w: npad_xl + P * (w + 1), :])

                    g = gp.tile([P, SLOTS, F], bf, tag="g")
                    for q in range(NQ):
                        off = q * QCAP
                        nc.gpsimd.dma_gather(
                            g[:, off // P: (off + QCAP) // P, :],
                            xcat[q * qrows: (q + 1) * qrows, :],
                            gx[:, off // 16: (off + QCAP) // 16],
                            QCAP, QCAP, F)

                    # scores
                    pp = ps_sc.tile([P, SLOTS * H], f32, tag="pp")
                    for s in range(NSUP):
                        # S^T[n, e] for this super-tile: broadcast the flat
                        # dst_rel row across partitions via PE, compare on DVE
                        dB = ps_db.tile([P, 512], f32, tag="dB")
                        nc.tensor.matmul(
                            out=dB[:], lhsT=ones_sb[:],
                            rhs=drf[:, 512 * s: 512 * (s + 1)],
                            start=True, stop=True)
                        st_s = mp.tile([P, 512], bf, tag="st_s")
                        nc.vector.tensor_tensor(
                            out=st_s[:], in0=dB[:],
                            in1=iotac_sb[:, 0:1].broadcast_to([P, 512]),
                            op=mybir.AluOpType.is_equal)
                        zt = ps_zt.tile([P, 512], f32, tag="zt")
                        for t in range(4):
                            e = 4 * s + t
                            nc.tensor.matmul(
                                out=zt[:, P * t: P * (t + 1)],
                                lhsT=g[:, e, :], rhs=ident_sb[:],
                                start=True, stop=False)
                            nc.tensor.matmul(
                                out=zt[:, P * t: P * (t + 1)],
                                lhsT=xr_w[:],
                                rhs=st_s[:, P * t: P * (t + 1)],
                                start=False, stop=True)
                        z_sb = mp.tile([P, 512], bf, tag="z_sb")
                        nc.scalar.copy(out=z_sb[:], in_=zt[:])
                        a_sb = mp.tile([P, 512], bf, tag="a_sb")
                        nc.scalar.activation(out=a_sb[:], in_=zt[:], func=Abs)
                        for t in range(4):
                            e = 4 * s + t
                            nc.tensor.matmul(
                                out=pp[:, H * e: H * (e + 1)],
                                lhsT=z_sb[:, P * t: P * (t + 1)],
                                rhs=attm_sb[:, :H], start=True, stop=False)
                            nc.tensor.matmul(
                                out=pp[:, H * e: H * (e + 1)],
                                lhsT=a_sb[:, P * t: P * (t + 1)],
                                rhs=attm_sb[:, H:], start=False, stop=True)
                    psb = mp.tile([P, SLOTS, H], bf, tag="psb")
                    nc.scalar.activation(
                        out=psb[:].rearrange("p a b -> p (a b)"), in_=pp[:],
                        func=Exp)

                    # aggregation — den rides as 4 extra rhs columns
                    pod = ps_od.tile([P, F + H], f32, tag="pod")
                    for s in range(NSUP):
                        S4 = mp.tile([P, 4, P], bf, tag="S4")
                        nc.vector.tensor_tensor(
                            out=S4[:], in0=iota_sb[:],
                            in1=dr[:, 4 * s: 4 * s + 4, None]
                            .broadcast_to([P, 4, P]),
                            op=mybir.AluOpType.is_equal)
                        xjp = mp.tile([P, 4, F + H], bf, tag="xjp")
                        nc.vector.tensor_tensor(
                            out=xjp[:, :, 0:F]
                            .rearrange("p t (h c) -> p t h c", h=H),
                            in0=g[:, 4 * s: 4 * s + 4, :]
                            .rearrange("p t (h c) -> p t h c", h=H),
                            in1=psb[:, 4 * s: 4 * s + 4, :, None]
                            .broadcast_to([P, 4, H, C]),
                            op=mybir.AluOpType.mult)
                        nc.vector.tensor_copy(
                            out=xjp[:, :, F:],
                            in_=psb[:, 4 * s: 4 * s + 4, :])
                        for t in range(4):
                            first = s == 0 and t == 0
                            last = s == NSUP - 1 and t == 3
                            nc.tensor.matmul(
                                out=pod[:], lhsT=S4[:, t, :],
                                rhs=xjp[:, t, :],
                                start=first, stop=last)

                    dn = fp.tile([P, H], f32, tag="dn")
                    nc.vector.tensor_scalar_add(out=dn[:], in0=pod[:, F:],
                                                scalar1=EPS)
                    rd = fp.tile([P, H], f32, tag="rd")
                    nc.vector.reciprocal(out=rd[:], in_=dn[:])
                    fin = fp.tile([P, H, C], f32, tag="fin")
                    nc.vector.tensor_tensor(
                        out=fin[:],
                        in0=pod[:, 0:F].rearrange("p (h c) -> p h c", h=H),
                        in1=rd[:, :, None].broadcast_to([P, H, C]),
                        op=mybir.AluOpType.mult)
                    fin2 = fp.tile([P, F], f32, tag="fin2")
                    nc.vector.tensor_tensor(
                        out=fin2[:],
                        in0=fin[:].rearrange("p h c -> p (h c)"),
                        in1=bias_sb[:], op=mybir.AluOpType.add)
                    nc.sync.dma_start(
                        out=outp[P * w: P * (w + 1), :], in_=fin2[:])

    nc.compile()
    return nc


# ------------------------------------------------------------------- driver


def _prepare(x, edge_index, W_l, W_r, att, bias):
    n_nodes = x.shape[0]
    n_edges = edge_index.shape[1]
    src = np.asarray(edge_index[0], np.int64)
    dst = np.asarray(edge_index[1], np.int64)
    order = np.argsort(dst, kind="stable")
    ssrc = src[order]
    sdst = dst[order]

    eb, nb = _split_cores(sdst, n_nodes, n_edges)
    npad_xl = _round_up(n_nodes, 2048)  # divisible by 4 quarters of 512-mult
    qrows = npad_xl // NQ
    assert qrows <= 32768

    cores = []
    for c in range(NCORES):
        nr_c = nb[c + 1] - nb[c]
        assert nr_c < 32000
        cores.append(_build_core_windows(
            ssrc[eb[c]:eb[c + 1]], sdst[eb[c]:eb[c + 1]], nb[c], nr_c, qrows))

    W = max(cd["wc"] for cd in cores)
    nr_pad = max(nb[c + 1] - nb[c] for c in range(NCORES))
    nrx = _round_up(max(W * P, 512), 512)

    xb = np.asarray(x, np.float32).astype(BF16)
    xT_np = np.zeros((P, npad_xl), BF16)
    xT_np[:, :n_nodes] = xb.T

    wl_np = np.asarray(W_l, np.float32).astype(BF16)
    wr_np = np.asarray(W_r, np.float32).astype(BF16)
    att_np = np.asarray(att, np.float32)
    attm_np = np.zeros((P, 2 * H), np.float32)
    for h in range(H):
        attm_np[C * h:C * (h + 1), h] = 0.6 * att_np[h]
        attm_np[C * h:C * (h + 1), H + h] = 0.4 * att_np[h]
    attm_np = attm_np.astype(BF16)
    bias_np = np.asarray(bias, np.float32)
    bias_bc_np = np.tile(bias_np[None, :], (P, 1)).astype(np.float32)

    in_maps = []
    for c in range(NCORES):
        cd = cores[c]
        wc = cd["wc"]
        nr_c = nb[c + 1] - nb[c]

        UW = EPW // 16
        gidx_np = np.zeros((W, P, UW), np.int16)
        drel_np = np.full((W, P, SLOTS), DUMMY_COL, np.float32)
        drelf_np = np.full((W, 1, EPW), DUMMY_COL, np.float32)

        for w in range(wc):
            # uidx/dstrel are [P, SLOTS] slot-shaped (slot j at
            # [j%128, j//128]); rewrap u for the int16 plane
            # (value j at [j%16, j//16]).
            uflat = cd["uidx"][w].T.reshape(-1)  # slot order: s*128+p
            gidx_np[w] = _wrap16(uflat, UW)
            drel_np[w] = cd["dstrel"][w]
            drelf_np[w, 0] = cd["dstrel"][w].T.reshape(-1)

        # window-major xr source: column 128*w + j = x[win_nb[w] + j]
        xrT_np = np.zeros((P, nrx), BF16)
        for w in range(wc):
            lo = int(cd["win_nb"][w])
            hi = min(lo + P, n_nodes)
            xrT_np[:, P * w: P * w + (hi - lo)] = xb[lo:hi].T

        in_maps.append({
            "xT": xT_np,
            "xrT": xrT_np,
            "Wl": wl_np,
            "Wr": wr_np,
            "attm": attm_np,
            "bias_bc": bias_bc_np,
            "gidx": gidx_np,
            "drel": drel_np.astype(BF16),
            "drelf": drelf_np.astype(BF16),
        })

    meta = dict(W=W, npad_xl=npad_xl, nrx=nrx, nr_pad=nr_pad, nb=nb,
                n_nodes=n_nodes, bias=bias_np, cores=cores,
                deg=np.bincount(dst, minlength=n_nodes))
    return in_maps, meta


_last_results = None


def kernel(x, edge_index, W_l, W_r, att, bias, _sim=False, _trace=False):
    global _last_results
    in_maps, meta = _prepare(x, edge_index, W_l, W_r, att, bias)
    nc = _build_nc(meta["W"], meta["npad_xl"], meta["nrx"], meta["nr_pad"])

    if _sim:
        from concourse.bass_interp import CoreSim
        results = []
        for c in range(NCORES):
            sim = CoreSim(nc, trace=False)
            for k, v in in_maps[c].items():
                sim.tensor(k)[:] = v
            sim.tensor("outp")[:] = 0.0
            sim.simulate()
            results.append({"outp": np.array(sim.tensor("outp"))})
    else:
        from concourse import bass_utils
        r = bass_utils.run_bass_kernel_spmd(
            nc, in_maps, core_ids=list(range(NCORES)), trace=_trace)
        _last_results = r
        results = r.results

    n_nodes = meta["n_nodes"]
    nb = meta["nb"]
    out = np.empty((n_nodes, F), np.float32)
    for c in range(NCORES):
        cd = meta["cores"][c]
        st = results[c]["outp"]
        for w in range(cd["wc"]):
            lo = int(cd["win_nb"][w])
            hi = int(cd["own_end"][w])
            out[lo:hi] = st[P * w: P * w + (hi - lo)]
    out[meta["deg"] == 0] = meta["bias"][None, :]
    return out
